# revision 4
# baseline (speedup 1.0000x reference)
"""Trainium2 Bass kernel for nn_EulerMisorientation3D (v2 redesign).

reference math (per voxel, Bunge ZXZ Euler angles scaled by [2pi, pi, 2pi]):
    tr  = sum_i g_ii * g_hat_ii          (elementwise diag product)
    out = mean( arccos(0.5*(tr-1))^2 )

Closed form used here (alpha=2pi*x0, beta=pi*x1, gamma=2pi*x2):
    u = cos(2pi*s), s = x0+x2;  v = cos(2pi*t), t = x0-x2;  c = cos(pi*x1)
    P4 = 4(1+z) = (1+c)(1+ch)(1+u*uh) + (1-c)(1-ch)(1+v*vh)
       = 2 + 2*c*ch + A*Ah + B*Bh,   A = u*(1+c), B = v*(1-c)
    theta = arccos(z) = pi/2 + 2*atan(tanh(0.25*(ln Q4 - ln P4))), Q4 = 8-P4
    loss = mean(theta^2)

Half-angle trick kills all range wraps: u = 1 - 2*sin^2(pi*s) with
sin(pi*s - pi) arg in [-pi, pi) (s in [0,2)), v = 1 - 2*sin^2(pi*t) with
pi*t in (-pi, pi), and c = -sin(pi*x1 - pi/2). So ACT does only plain sins.

Engine split:
    PE    s = x0+x2 via two identity-weight fp32r matmuls accumulating in
          PSUM (weights are exactly +-1 so the adds are near-exact).
    Pool  t = x0-x2 (tensor_tensor), cc = c*ch chain, G = 2cc+AA+BB assembly.
    ACT   6 sins/voxel (s,t from PSUM/SBUF, beta), then lnP, lnQ, tanh,
          atan.  Queue pinned so table sets load once each:
          trig_and_small (sin) -> natural_log (ln) -> sigmoid_and_others
          (tanh+arctan).
    DVE   bf16 product chain (q=sigma^2, uv=1-2q, cp/cm=1-+sigma_b, AB,
          AABB, S1), G clamp, d = lnQ-lnP, theta^2 square+reduce
          (tensor_tensor_reduce with accumulate).

Sharding: flattened voxel axis (2097152) split over 8 cores; each core
reduces its 262144 voxels to acc[128, SPANS] partial sums; host sums (f64)
and divides by N.
"""

import math

import numpy as np

import concourse.bacc as bacc
import concourse.tile as tile
from concourse.tile_rust import add_dep_helper
from concourse import mybir
from concourse.bass_utils import run_bass_kernel_spmd

F32 = mybir.dt.float32
F32R = mybir.dt.float32r
BF16 = mybir.dt.bfloat16
AF = mybir.ActivationFunctionType
OP = mybir.AluOpType

N_CORES = 8
NVOX = 128 * 128 * 128          # 2097152 voxels
PER = NVOX // N_CORES           # 262144 voxels per core
P = 128                         # SBUF partitions
COLS = PER // P                 # 2048 free-dim columns per core
T = 4                           # tiles
C = COLS // T                   # 512 columns per tile
SPANS = T // 2                  # tail processed in 2-tile spans

PI = math.pi
LN_EPS = 5e-5
GMAX = 5.99609375               # bf16-exact clamp so 6-G stays positive


def build_bass():
    nc = bacc.Bacc("TRN2", target_bir_lowering=False, debug=False,
                   num_devices=N_CORES)
    xs = nc.declare_dram_parameter("xs", [3, PER], F32R, isOutput=False)
    xh = nc.declare_dram_parameter("xh", [3, PER], F32R, isOutput=False)
    wid = nc.declare_dram_parameter("wi", [P, P], F32R, isOutput=False)
    out = nc.declare_dram_parameter("o", [P, SPANS], F32, isOutput=True)

    xs_v = xs[:].rearrange("c (p q) -> p c q", p=P)   # [128, 3, 2048]
    xh_v = xh[:].rearrange("c (p q) -> p c q", p=P)

    with tile.TileContext(nc) as tc:
        with (
            tc.tile_pool(name="io", bufs=2) as io,
            tc.tile_pool(name="wk", bufs=2) as wk,
            tc.tile_pool(name="bt", bufs=2) as bt,
            tc.tile_pool(name="tl", bufs=1) as tl,
            tc.tile_pool(name="big", bufs=1) as big,
            tc.psum_pool(name="ps", bufs=3) as ps,
        ):
            acc = big.tile([P, SPANS], F32, tag="acc")
            b_mpi = big.tile([P, 1], F32, tag="b_mpi")
            b_mpi2 = big.tile([P, 1], F32, tag="b_mpi2")
            b_p2 = big.tile([P, 1], F32, tag="b_p2")
            b_p6 = big.tile([P, 1], F32, tag="b_p6")
            nc.vector.memset(b_mpi, -PI)
            nc.vector.memset(b_mpi2, -PI / 2)
            nc.vector.memset(b_p2, 2.0 + LN_EPS)
            nc.vector.memset(b_p6, 6.0 + LN_EPS)
            wi = big.tile([P, P], F32R, tag="wi")
            nc.sync.dma_start(out=wi, in_=wid[:])

            # ---- input DMAs.  per tile: in4 = [x0x|x2x|x0h|x2h];
            # per 2-tile span: xb = [x1x|x1h] (1024 cols)
            in4s, xbs = [], []
            for j in range(T):
                in4 = io.tile([P, 2, 2, C], F32R, tag="in4")
                in4s.append(in4)
            for sp in range(SPANS):
                xb = bt.tile([P, 2, 2 * C], F32R, tag="xb")
                xbs.append(xb)
            for j in range(T):
                sl = slice(j * C, (j + 1) * C)
                nc.sync.dma_start(out=in4s[j][:, 0, :, :],
                                  in_=xs_v[:, 0:3:2, sl])
                nc.sync.dma_start(out=in4s[j][:, 1, :, :],
                                  in_=xh_v[:, 0:3:2, sl])
                if j % 2 == 1:
                    sp = j // 2
                    sl2 = slice(sp * 2 * C, (sp + 1) * 2 * C)
                    nc.sync.dma_start(out=xbs[sp][:, 0, :],
                                      in_=xs_v[:, 1, sl2])
                    nc.sync.dma_start(out=xbs[sp][:, 1, :],
                                      in_=xh_v[:, 1, sl2])

            act_chain = []   # pinned ACT order (table-set friendly)
            sgs, tsbs, sbs, cpcms, ccs = [], [], [], [], []
            s1s, gcs = [], []

            # ---- beta sins + cc chain per span (early: only needs xb)
            for sp in range(SPANS):
                sb = bt.tile([P, 2, 2 * C], BF16, tag="sb")
                act_chain.append(nc.scalar.activation(
                    sb[:], xbs[sp][:].bitcast(F32), AF.Sin,
                    bias=b_mpi2[:], scale=PI))
                sbs.append(sb)
                # cp/cm: [P, cp|cm, x|h, 2C]
                cpcm = bt.tile([P, 2, 2, 2 * C], BF16, tag="cpcm")
                nc.vector.tensor_scalar(cpcm[:, 0, :, :], sb[:], -1.0, 1.0,
                                        OP.mult, OP.add)
                nc.vector.tensor_scalar(cpcm[:, 1, :, :], sb[:], 1.0, None,
                                        OP.add)
                cpcms.append(cpcm)
                # cc = c*ch ; ccc = 2cc   (pool)
                cc = bt.tile([P, 2 * C], BF16, tag="cc")
                nc.gpsimd.tensor_tensor(cc[:], sb[:, 0, :], sb[:, 1, :],
                                        OP.mult)
                nc.gpsimd.tensor_tensor(cc[:], cc[:], cc[:], OP.add)
                ccs.append(cc)

            # ---- per tile: PE s-adds, pool t-adds, sins, product chain
            for j in range(T):
                in4 = in4s[j]
                # s = x0+x2 for both tensors -> PSUM [P, x|h, C]
                pst = ps.tile([P, 2, C], F32, tag="pst")
                for k in range(2):
                    nc.tensor.matmul(pst[:, k, :], wi[:], in4[:, k, 0, :],
                                     start=True, stop=False)
                    nc.tensor.matmul(pst[:, k, :], wi[:], in4[:, k, 1, :],
                                     start=False, stop=True)
                # t = x0-x2 (pool) -> SBUF f32
                tsb = wk.tile([P, 2, C], F32, tag="tsb")
                nc.gpsimd.tensor_sub(tsb[:], in4[:, :, 0, :].bitcast(F32),
                                     in4[:, :, 1, :].bitcast(F32))
                tsbs.append(tsb)

                # sins: sg = [sig_s_x|sig_s_h|sig_t_x|sig_t_h] bf16
                sg = wk.tile([P, 2, 2, C], BF16, tag="sg")
                act_chain.append(nc.scalar.activation(
                    sg[:, 0], pst[:], AF.Sin, bias=b_mpi[:], scale=PI))
                act_chain.append(nc.scalar.activation(
                    sg[:, 1], tsb[:], AF.Sin, bias=0.0, scale=PI))
                sgs.append(sg)

                # DVE product chain (bf16)
                qq = wk.tile([P, 2, 2, C], BF16, tag="qq")
                nc.vector.tensor_mul(qq[:], sg[:], sg[:])
                uv = wk.tile([P, 2, 2, C], BF16, tag="uv")
                nc.vector.tensor_scalar(uv[:], qq[:], -2.0, 1.0,
                                        OP.mult, OP.add)
                # AB = uv * cpcm-slice   [P, u|v, x|h, C]
                sp, half = j // 2, j % 2
                csl = slice(half * C, (half + 1) * C)
                ab = wk.tile([P, 2, 2, C], BF16, tag="ab")
                nc.vector.tensor_mul(ab[:], uv[:], cpcms[sp][:, :, :, csl])
                # AABB = A*Ah | B*Bh ; S1 = AA+BB
                aabb = wk.tile([P, 2, C], BF16, tag="aabb")
                nc.vector.tensor_mul(aabb[:], ab[:, :, 0, :], ab[:, :, 1, :])
                s1 = wk.tile([P, C], BF16, tag="s1")
                nc.vector.tensor_add(s1[:], aabb[:, 0, :], aabb[:, 1, :])
                s1s.append(s1)

            # ---- G assembly (pool) + clamp (DVE) per tile into span bufs
            gbufs = []
            for sp in range(SPANS):
                gbuf = tl.tile([P, 2, C], BF16, tag=f"gbuf{sp}")
                gbufs.append(gbuf)
            for j in range(T):
                sp, half = j // 2, j % 2
                csl = slice(half * C, (half + 1) * C)
                nc.gpsimd.tensor_tensor(gbufs[sp][:, half, :], ccs[sp][:, csl],
                                        s1s[j][:], OP.add)
            for sp in range(SPANS):
                # clamp G <= GMAX so Q4 = 6+eps-G stays positive in bf16
                nc.vector.tensor_scalar(gbufs[sp][:], gbufs[sp][:], GMAX,
                                        None, OP.min)

            # ---- tail per span: lnP, lnQ (ACT) ; d (DVE) ; tanh, atan
            lnps, lnqs, ds, ws, a_s = [], [], [], [], []
            for sp in range(SPANS):
                lnp = tl.tile([P, 2 * C], BF16, tag=f"lnp{sp}")
                act_chain.append(nc.scalar.activation(
                    lnp[:], gbufs[sp][:], AF.Ln, bias=b_p2[:], scale=1.0))
                lnps.append(lnp)
                lnq = tl.tile([P, 2 * C], BF16, tag=f"lnq{sp}")
                act_chain.append(nc.scalar.activation(
                    lnq[:], gbufs[sp][:], AF.Ln, bias=b_p6[:], scale=-1.0))
                lnqs.append(lnq)
            for sp in range(SPANS):
                d = tl.tile([P, 2 * C], BF16, tag=f"d{sp}")
                nc.vector.tensor_sub(d[:], lnqs[sp][:], lnps[sp][:])
                ds.append(d)
            for sp in range(SPANS):
                w = tl.tile([P, 2 * C], BF16, tag=f"w{sp}")
                act_chain.append(nc.scalar.activation(
                    w[:], ds[sp][:], AF.Tanh, bias=0.0, scale=0.25))
                ws.append(w)
            for sp in range(SPANS):
                a = tl.tile([P, 2 * C], BF16, tag=f"a{sp}")
                act_chain.append(nc.scalar.activation(
                    a[:], ws[sp][:], AF.Arctan))
                a_s.append(a)
            # theta = pi/2 + 2a ; accumulate theta^2
            for sp in range(SPANS):
                t1 = tl.tile([P, 2 * C], F32, tag=f"t1{sp}")
                nc.vector.tensor_scalar(t1[:], a_s[sp][:], 2.0, PI / 2,
                                        OP.mult, OP.add)
                scr = tl.tile([P, 2 * C], F32, tag=f"scr{sp}")
                nc.vector.tensor_mul(scr[:], t1[:], t1[:])
                nc.vector.reduce_sum(acc[:, sp:sp + 1], scr[:],
                                     axis=mybir.AxisListType.X)

            # pin ACT queue order for table-set locality
            for ai, bi in zip(act_chain, act_chain[1:]):
                add_dep_helper(bi.ins, ai.ins, sync=False,
                               reason="ACT table-set ordering")

            nc.sync.dma_start(out=out[:], in_=acc[:])

    nc.compile()
    return nc


_CACHE = {}


def _get_nc():
    if "nc" not in _CACHE:
        _CACHE["nc"] = build_bass()
    return _CACHE["nc"]


def _run(x, x_hat, **spmd_kwargs):
    x = np.ascontiguousarray(np.asarray(x, dtype=np.float32).reshape(3, NVOX))
    xh = np.ascontiguousarray(
        np.asarray(x_hat, dtype=np.float32).reshape(3, NVOX))
    wi = np.eye(P, dtype=np.float32)

    in_maps = []
    for c in range(N_CORES):
        sl = slice(c * PER, (c + 1) * PER)
        in_maps.append({
            "xs": np.ascontiguousarray(x[:, sl]),
            "xh": np.ascontiguousarray(xh[:, sl]),
            "wi": wi,
        })

    nc = _get_nc()
    res = run_bass_kernel_spmd(
        nc, in_maps, core_ids=list(range(N_CORES)), **spmd_kwargs)
    total = 0.0
    for r in res.results:
        total += r["o"].astype(np.float64).sum()
    return np.float32(total / NVOX), res


def kernel(x: np.ndarray, x_hat: np.ndarray) -> np.ndarray:
    val, _ = _run(x, x_hat)
    return val


# revision 6
# speedup vs baseline: 1.1780x; 1.1780x over previous
"""Trainium2 Bass kernel for nn_EulerMisorientation3D (v2).

reference math (per voxel, Bunge ZXZ Euler angles scaled by [2pi, pi, 2pi]):
    tr  = sum_i g_ii * g_hat_ii          (elementwise diag product)
    out = mean( arccos(0.5*(tr-1))^2 )

Closed form (alpha=2pi*x0, beta=pi*x1, gamma=2pi*x2):
    u = cos(2pi*s), s = x0+x2;  v = cos(2pi*t), t = x0-x2;  c = cos(pi*x1)
    P4 = 4(1+z) = 2 + 2*c*ch + A*Ah + B*Bh,  A = u*(1+c), B = v*(1-c)
    r  = sqrt(Q4/P4) = tan(theta/2),  Q4 = 8-P4 = 6-G,  G = P4-2
    theta = 2*atan(r),  r = exp(0.5*(ln Q4 - ln P4));  loss = mean(theta^2)

Half-angle trick kills all range wraps: u = 1-2*sin^2(pi*s) with
sin(pi*s-pi) arg within the sin spline domain even for x_hat noise
(|arg| <= ~1.24pi < 4), v = 1-2*sin^2(pi*t), c = -sin(pi*x1 - pi/2).

Engine split:
    PE    s = x0+x2 per tile via two identity-weight fp32r matmuls
          accumulating in PSUM (weights exactly 1 -> near-exact adds).
    Pool  t = x0-x2 per tile; cc = c*ch per span.
    ACT   6 sins/voxel then lnP, lnQ, exp(d/2), atan per span.  Queue is
          pinned in data-arrival order and grouped by table set:
          sin* (trig_and_small) -> ln* (natural_log) -> exp* -> atan*.
    DVE   bf16 chain with flat contiguous APs (2x/4x modes): q=sg^2,
          uv=1-2q, cp/cm, AB, AABB, S1, G=2cc+S1, clamp, d=lnQ-lnP,
          a^2, reduce.  theta^2 = 4*atan(r)^2; host multiplies by 4.

Sharding: voxel axis split over 8 cores; per-core acc[128, SPANS] partial
sums of atan(r)^2; host sums (f64) * 4 / N.
"""

import math

import numpy as np

import concourse.bacc as bacc
import concourse.tile as tile
from concourse.tile_rust import add_dep_helper
from concourse import mybir
from concourse.bass_utils import run_bass_kernel_spmd

F32 = mybir.dt.float32
F32R = mybir.dt.float32r
BF16 = mybir.dt.bfloat16
AF = mybir.ActivationFunctionType
OP = mybir.AluOpType

N_CORES = 8
NVOX = 128 * 128 * 128          # 2097152 voxels
PER = NVOX // N_CORES           # 262144 voxels per core
P = 128
COLS = PER // P                 # 2048
T = 4                           # tiles
C = COLS // T                   # 512
SPANS = T // 2                  # spans of 2 tiles (1024 cols)
W = 2 * C                       # span width

PI = math.pi
LN_EPS = 5e-5
GMAX = 5.99609375               # bf16-exact clamp: Q4 = 6+eps-G > 0


def build_bass():
    nc = bacc.Bacc("TRN2", target_bir_lowering=False, debug=False,
                   num_devices=N_CORES)
    xs = nc.declare_dram_parameter("xs", [3, PER], F32R, isOutput=False)
    xh = nc.declare_dram_parameter("xh", [3, PER], F32R, isOutput=False)
    wid = nc.declare_dram_parameter("wi", [P, P], F32R, isOutput=False)
    out = nc.declare_dram_parameter("o", [P, SPANS], F32, isOutput=True)

    xs_v = xs[:].rearrange("c (p q) -> p c q", p=P)   # [128, 3, 2048]
    xh_v = xh[:].rearrange("c (p q) -> p c q", p=P)

    with tile.TileContext(nc) as tc:
        with (
            tc.tile_pool(name="io", bufs=2) as io,
            tc.tile_pool(name="wk", bufs=2) as wk,
            tc.tile_pool(name="bt", bufs=2) as bt,
            tc.tile_pool(name="tl", bufs=1) as tl,
            tc.tile_pool(name="big", bufs=1) as big,
            tc.psum_pool(name="ps", bufs=3) as ps,
        ):
            acc = big.tile([P, SPANS], F32, tag="acc")
            b_mpi = big.tile([P, 1], F32, tag="b_mpi")
            b_mpi2 = big.tile([P, 1], F32, tag="b_mpi2")
            b_p2 = big.tile([P, 1], F32, tag="b_p2")
            b_p6 = big.tile([P, 1], F32, tag="b_p6")
            nc.vector.memset(b_mpi, -PI)
            nc.vector.memset(b_mpi2, -PI / 2)
            nc.vector.memset(b_p2, 2.0 + LN_EPS)
            nc.vector.memset(b_p6, 6.0 + LN_EPS)
            wi = big.tile([P, P], F32R, tag="wi")
            nc.sync.dma_start(out=wi, in_=wid[:])

            # ---- tiles (allocated up-front; DMAs in arrival order)
            in4s = [io.tile([P, 2, 2, C], F32R, tag="in4", name=f"in4_{j}")
                    for j in range(T)]
            xbs = [bt.tile([P, 2, W], F32R, tag="xb", name=f"xb_{s}")
                   for s in range(SPANS)]

            def dma_tile(j):
                sl = slice(j * C, (j + 1) * C)
                nc.sync.dma_start(out=in4s[j][:, 0], in_=xs_v[:, 0:3:2, sl])
                nc.sync.dma_start(out=in4s[j][:, 1], in_=xh_v[:, 0:3:2, sl])

            def dma_beta(s):
                sl = slice(s * W, (s + 1) * W)
                nc.sync.dma_start(out=xbs[s][:, 0, :], in_=xs_v[:, 1, sl])
                nc.sync.dma_start(out=xbs[s][:, 1, :], in_=xh_v[:, 1, sl])

            dma_tile(0)
            dma_beta(0)
            dma_tile(1)
            dma_beta(1)
            dma_tile(2)
            dma_tile(3)

            # per-span tiles
            sgs = [wk.tile([P, 2, 2, W], BF16, tag="sg", name=f"sg_{s}")
                   for s in range(SPANS)]       # [x|h][sig_s|sig_t]
            sbs = [bt.tile([P, 2, W], BF16, tag="sb", name=f"sb_{s}")
                   for s in range(SPANS)]
            ccs = [bt.tile([P, W], BF16, tag="cc", name=f"cc_{s}")
                   for s in range(SPANS)]
            cpcms = [bt.tile([P, 2, 2, W], BF16, tag="cpcm",
                             name=f"cpcm_{s}") for s in range(SPANS)]

            sin_list = []    # ACT sins per tile, in emit order
            tail_chain = []  # ACT tail, grouped by table set

            # ---- per tile: PE s-adds -> psum; pool t-adds -> sbuf
            for j in range(T):
                in4 = in4s[j]
                pst = ps.tile([P, 2, C], F32, tag="pst")
                for k in range(2):
                    nc.tensor.matmul(pst[:, k, :], wi[:], in4[:, k, 0, :],
                                     start=True, stop=False)
                    nc.tensor.matmul(pst[:, k, :], wi[:], in4[:, k, 1, :],
                                     start=False, stop=True)
                tsb = wk.tile([P, 2, C], F32, tag="tsb")
                nc.gpsimd.tensor_sub(tsb[:], in4[:, :, 0, :].bitcast(F32),
                                     in4[:, :, 1, :].bitcast(F32))

                sp, h = j // 2, j % 2
                csl = slice(h * C, (h + 1) * C)
                sg = sgs[sp]
                sin_list.append(nc.scalar.activation(
                    sg[:, :, 0, csl], pst[:], AF.Sin,
                    bias=b_mpi[:], scale=PI))
                sin_list.append(nc.scalar.activation(
                    sg[:, :, 1, csl], tsb[:], AF.Sin, bias=0.0, scale=PI))

            # beta sins (+ pool cc) per span
            beta_sins = []
            for s in range(SPANS):
                ins = nc.scalar.activation(
                    sbs[s][:], xbs[s][:].bitcast(F32), AF.Sin,
                    bias=b_mpi2[:], scale=PI)
                beta_sins.append(ins)
                nc.gpsimd.tensor_tensor(ccs[s][:], sbs[s][:, 0, :],
                                        sbs[s][:, 1, :], OP.mult)

            # ACT sin order: b0, s0, t0, s1, t1, b1, s2, t2, s3, t3
            order = [beta_sins[0], sin_list[0], sin_list[1],
                     sin_list[2], sin_list[3], beta_sins[1],
                     sin_list[4], sin_list[5], sin_list[6], sin_list[7]]

            # ---- per span: DVE product chain (flat APs for 2x/4x modes)
            gbufs = []
            for s in range(SPANS):
                sb, sg, cpcm = sbs[s], sgs[s], cpcms[s]
                for k in range(2):
                    nc.vector.tensor_scalar(cpcm[:, k, 0, :], sb[:, k, :],
                                            -1.0, 1.0, OP.mult, OP.add)
                    nc.vector.tensor_scalar(cpcm[:, k, 1, :], sb[:, k, :],
                                            1.0, None, OP.add)
                fl4 = lambda ap: ap.rearrange("p a b w -> p (a b w)")
                fl3 = lambda ap: ap.rearrange("p a w -> p (a w)")
                qq = wk.tile([P, 2, 2, W], BF16, tag="qq")
                nc.vector.tensor_mul(fl4(qq[:]), fl4(sg[:]), fl4(sg[:]))
                uv = wk.tile([P, 2, 2, W], BF16, tag="uv")
                nc.vector.tensor_scalar(fl4(uv[:]), fl4(qq[:]),
                                        -2.0, 1.0, OP.mult, OP.add)
                ab = wk.tile([P, 2, 2, W], BF16, tag="ab")
                nc.vector.tensor_mul(fl4(ab[:]), fl4(uv[:]), fl4(cpcm[:]))
                aabb = wk.tile([P, 2, W], BF16, tag="aabb")
                nc.vector.tensor_mul(fl3(aabb[:]),
                                     fl3(ab[:, 0]), fl3(ab[:, 1]))
                s1 = wk.tile([P, W], BF16, tag="s1")
                nc.vector.tensor_add(s1[:], aabb[:, 0, :], aabb[:, 1, :])
                g = tl.tile([P, W], BF16, tag=f"g{s}")
                nc.vector.scalar_tensor_tensor(g[:], ccs[s][:], 2.0, s1[:],
                                               OP.mult, OP.add)
                nc.vector.tensor_scalar(g[:], g[:], GMAX, None, OP.min)
                gbufs.append(g)

            # ---- tail: lns (one table set), exps, atans; d/sq/red on DVE
            lnps, lnqs, ds, rs, a_s = [], [], [], [], []
            for s in range(SPANS):
                lnp = tl.tile([P, W], BF16, tag=f"lnp{s}")
                tail_chain.append(nc.scalar.activation(
                    lnp[:], gbufs[s][:], AF.Ln, bias=b_p2[:], scale=1.0))
                lnps.append(lnp)
                lnq = tl.tile([P, W], BF16, tag=f"lnq{s}")
                tail_chain.append(nc.scalar.activation(
                    lnq[:], gbufs[s][:], AF.Ln, bias=b_p6[:], scale=-1.0))
                lnqs.append(lnq)
            for s in range(SPANS):
                d = tl.tile([P, W], BF16, tag=f"d{s}")
                nc.vector.tensor_sub(d[:], lnqs[s][:], lnps[s][:])
                ds.append(d)
            for s in range(SPANS):
                r = tl.tile([P, W], BF16, tag=f"r{s}")
                tail_chain.append(nc.scalar.activation(
                    r[:], ds[s][:], AF.Exp, bias=0.0, scale=0.5))
                rs.append(r)
            for s in range(SPANS):
                a = tl.tile([P, W], BF16, tag=f"a{s}")
                tail_chain.append(nc.scalar.activation(
                    a[:], rs[s][:], AF.Arctan))
                a_s.append(a)
            for s in range(SPANS):
                sq = tl.tile([P, W], F32, tag=f"sq{s}")
                nc.vector.tensor_mul(sq[:], a_s[s][:], a_s[s][:])
                nc.vector.reduce_sum(acc[:, s:s + 1], sq[:],
                                     axis=mybir.AxisListType.X)

            # pin ACT order: sins (arrival) then ln*, exp*, atan* groups
            full = order + tail_chain
            for ai, bi in zip(full, full[1:]):
                add_dep_helper(bi.ins, ai.ins, sync=False,
                               reason="ACT table-set ordering")

            nc.sync.dma_start(out=out[:], in_=acc[:])

    nc.compile()
    return nc


_CACHE = {}


def _get_nc():
    if "nc" not in _CACHE:
        _CACHE["nc"] = build_bass()
    return _CACHE["nc"]


def _run(x, x_hat, **spmd_kwargs):
    x = np.ascontiguousarray(np.asarray(x, dtype=np.float32).reshape(3, NVOX))
    xh = np.ascontiguousarray(
        np.asarray(x_hat, dtype=np.float32).reshape(3, NVOX))
    wi = np.eye(P, dtype=np.float32)

    in_maps = []
    for c in range(N_CORES):
        sl = slice(c * PER, (c + 1) * PER)
        in_maps.append({
            "xs": np.ascontiguousarray(x[:, sl]),
            "xh": np.ascontiguousarray(xh[:, sl]),
            "wi": wi,
        })

    nc = _get_nc()
    res = run_bass_kernel_spmd(
        nc, in_maps, core_ids=list(range(N_CORES)), **spmd_kwargs)
    total = 0.0
    for r in res.results:
        total += r["o"].astype(np.float64).sum()
    return np.float32(4.0 * total / NVOX), res


def kernel(x: np.ndarray, x_hat: np.ndarray) -> np.ndarray:
    val, _ = _run(x, x_hat)
    return val


# revision 8
# speedup vs baseline: 1.3435x; 1.1405x over previous
"""Trainium2 Bass kernel for nn_EulerMisorientation3D (v2).

reference math (per voxel, Bunge ZXZ Euler angles scaled by [2pi, pi, 2pi]):
    tr  = sum_i g_ii * g_hat_ii          (elementwise diag product)
    out = mean( arccos(0.5*(tr-1))^2 )

Closed form (alpha=2pi*x0, beta=pi*x1, gamma=2pi*x2):
    u = cos(2pi*s), s = x0+x2;  v = cos(2pi*t), t = x0-x2;  c = cos(pi*x1)
    P4 = 4(1+z) = 2 + 2*c*ch + A*Ah + B*Bh,  A = u*(1+c), B = v*(1-c)
    r  = sqrt(Q4/P4) = tan(theta/2),  Q4 = 8-P4 = 6-G,  G = P4-2
    theta = 2*atan(r),  r = exp(0.5*(ln Q4 - ln P4));  loss = mean(theta^2)

Half-angle trick kills all range wraps: u = 1-2*sin^2(pi*s) with
sin(pi*s-pi) arg within the sin spline domain even for x_hat noise
(|arg| <= ~1.24pi < 4), v = 1-2*sin^2(pi*t), c = -sin(pi*x1 - pi/2).

Engine split:
    PE    s = x0+x2 per tile via two identity-weight fp32r matmuls
          accumulating in PSUM (weights exactly 1 -> near-exact adds).
    Pool  t = x0-x2 per tile; cc = c*ch per span.
    ACT   6 sins/voxel then lnP, lnQ, exp(d/2), atan per span.  Queue is
          pinned in data-arrival order and grouped by table set:
          sin* (trig_and_small) -> ln* (natural_log) -> exp* -> atan*.
    DVE   bf16 chain with flat contiguous APs (2x/4x modes): q=sg^2,
          uv=1-2q, cp/cm, AB, AABB, S1, G=2cc+S1, clamp, d=lnQ-lnP,
          a^2, reduce.  theta^2 = 4*atan(r)^2; host multiplies by 4.

Sharding: voxel axis split over 8 cores; per-core acc[128, SPANS] partial
sums of atan(r)^2; host sums (f64) * 4 / N.
"""

import math

import numpy as np

import concourse.bacc as bacc
import concourse.tile as tile
from concourse.tile_rust import add_dep_helper
from concourse import mybir
from concourse.bass_utils import run_bass_kernel_spmd

F32 = mybir.dt.float32
F32R = mybir.dt.float32r
BF16 = mybir.dt.bfloat16
AF = mybir.ActivationFunctionType
OP = mybir.AluOpType

N_CORES = 8
NVOX = 128 * 128 * 128          # 2097152 voxels
PER = NVOX // N_CORES           # 262144 voxels per core
P = 128
COLS = PER // P                 # 2048
T = 4                           # tiles
C = COLS // T                   # 512
SPANS = T // 2                  # spans of 2 tiles (1024 cols)
W = 2 * C                       # span width

PI = math.pi
LN_EPS = 5e-5
GMAX = 5.99609375               # bf16-exact clamp: Q4 = 6+eps-G > 0


def build_bass():
    nc = bacc.Bacc("TRN2", target_bir_lowering=False, debug=False,
                   num_devices=N_CORES)
    xs = nc.declare_dram_parameter("xs", [3, PER], F32R, isOutput=False)
    xh = nc.declare_dram_parameter("xh", [3, PER], F32R, isOutput=False)
    wid = nc.declare_dram_parameter("wi", [P, P], F32R, isOutput=False)
    wnid = nc.declare_dram_parameter("wni", [P, P], F32R, isOutput=False)
    out = nc.declare_dram_parameter("o", [P, SPANS], F32, isOutput=True)

    xs_v = xs[:].rearrange("c (p q) -> p c q", p=P)   # [128, 3, 2048]
    xh_v = xh[:].rearrange("c (p q) -> p c q", p=P)

    with tile.TileContext(nc) as tc:
        with (
            tc.tile_pool(name="io", bufs=2) as io,
            tc.tile_pool(name="wk", bufs=2) as wk,
            tc.tile_pool(name="bt", bufs=2) as bt,
            tc.tile_pool(name="tl", bufs=1) as tl,
            tc.tile_pool(name="big", bufs=1) as big,
            tc.psum_pool(name="ps", bufs=2) as ps,
        ):
            acc = big.tile([P, SPANS], F32, tag="acc")
            b_mpi = big.tile([P, 1], F32, tag="b_mpi")
            b_mpi2 = big.tile([P, 1], F32, tag="b_mpi2")
            b_p2 = big.tile([P, 1], F32, tag="b_p2")
            b_p6 = big.tile([P, 1], F32, tag="b_p6")
            nc.vector.memset(b_mpi, -PI)
            nc.vector.memset(b_mpi2, -PI / 2)
            nc.vector.memset(b_p2, 2.0 + LN_EPS)
            nc.vector.memset(b_p6, 6.0 + LN_EPS)
            wi = big.tile([P, P], F32R, tag="wi")
            nc.sync.dma_start(out=wi, in_=wid[:])
            wni = big.tile([P, P], F32R, tag="wni")
            nc.sync.dma_start(out=wni, in_=wnid[:])

            # ---- tiles (allocated up-front; DMAs in arrival order)
            in4s = [io.tile([P, 2, 2, C], F32R, tag="in4", name=f"in4_{j}")
                    for j in range(T)]
            xbs = [bt.tile([P, 2, W], F32R, tag="xb", name=f"xb_{s}")
                   for s in range(SPANS)]

            def dma_tile(j):
                sl = slice(j * C, (j + 1) * C)
                nc.sync.dma_start(out=in4s[j][:, 0], in_=xs_v[:, 0:3:2, sl])
                nc.sync.dma_start(out=in4s[j][:, 1], in_=xh_v[:, 0:3:2, sl])

            def dma_beta(s):
                sl = slice(s * W, (s + 1) * W)
                nc.sync.dma_start(out=xbs[s][:, 0, :], in_=xs_v[:, 1, sl])
                nc.sync.dma_start(out=xbs[s][:, 1, :], in_=xh_v[:, 1, sl])

            dma_tile(0)
            dma_beta(0)
            dma_tile(1)
            dma_beta(1)
            dma_tile(2)
            dma_tile(3)

            # per-span tiles
            sgs = [wk.tile([P, 2, 2, W], BF16, tag="sg", name=f"sg_{s}")
                   for s in range(SPANS)]       # [x|h][sig_s|sig_t]
            sbs = [bt.tile([P, 2, W], BF16, tag="sb", name=f"sb_{s}")
                   for s in range(SPANS)]
            ccs = [bt.tile([P, W], BF16, tag="cc", name=f"cc_{s}")
                   for s in range(SPANS)]
            cpcms = [bt.tile([P, 2, 2, W], BF16, tag="cpcm",
                             name=f"cpcm_{s}") for s in range(SPANS)]

            sin_list = []    # ACT sins per tile, in emit order
            tail_chain = []  # ACT tail, grouped by table set

            # ---- per tile: all adds on PE (s and t) -> psum
            for j in range(T):
                in4 = in4s[j]
                pst = ps.tile([P, 2, C], F32, tag="pst")
                ptt = ps.tile([P, 2, C], F32, tag="ptt")
                for k in range(2):
                    nc.tensor.matmul(pst[:, k, :], wi[:], in4[:, k, 0, :],
                                     start=True, stop=False)
                    nc.tensor.matmul(pst[:, k, :], wi[:], in4[:, k, 1, :],
                                     start=False, stop=True)
                    nc.tensor.matmul(ptt[:, k, :], wi[:], in4[:, k, 0, :],
                                     start=True, stop=False)
                    nc.tensor.matmul(ptt[:, k, :], wni[:], in4[:, k, 1, :],
                                     start=False, stop=True)

                sp, h = j // 2, j % 2
                csl = slice(h * C, (h + 1) * C)
                sg = sgs[sp]
                sin_list.append(nc.scalar.activation(
                    sg[:, :, 0, csl], pst[:], AF.Sin,
                    bias=b_mpi[:], scale=PI))
                sin_list.append(nc.scalar.activation(
                    sg[:, :, 1, csl], ptt[:], AF.Sin, bias=0.0, scale=PI))

            # beta sins per span
            beta_sins = []
            for s in range(SPANS):
                ins = nc.scalar.activation(
                    sbs[s][:], xbs[s][:].bitcast(F32), AF.Sin,
                    bias=b_mpi2[:], scale=PI)
                beta_sins.append(ins)

            # ACT sin order: b0, s0, t0, s1, t1, b1, s2, t2, s3, t3
            order = [beta_sins[0], sin_list[0], sin_list[1],
                     sin_list[2], sin_list[3], beta_sins[1],
                     sin_list[4], sin_list[5], sin_list[6], sin_list[7]]

            # ---- per span: DVE product chain (flat APs for 2x/4x modes)
            gbufs = []
            for s in range(SPANS):
                sb, sg, cpcm = sbs[s], sgs[s], cpcms[s]
                for k in range(2):
                    nc.vector.tensor_scalar(cpcm[:, k, 0, :], sb[:, k, :],
                                            -1.0, 1.0, OP.mult, OP.add)
                    nc.vector.tensor_scalar(cpcm[:, k, 1, :], sb[:, k, :],
                                            1.0, None, OP.add)
                nc.vector.tensor_mul(ccs[s][:], sb[:, 0, :], sb[:, 1, :])
                fl4 = lambda ap: ap.rearrange("p a b w -> p (a b w)")
                fl3 = lambda ap: ap.rearrange("p a w -> p (a w)")
                qq = wk.tile([P, 2, 2, W], BF16, tag="qq")
                nc.vector.tensor_mul(fl4(qq[:]), fl4(sg[:]), fl4(sg[:]))
                uv = wk.tile([P, 2, 2, W], BF16, tag="uv")
                nc.vector.tensor_scalar(fl4(uv[:]), fl4(qq[:]),
                                        -2.0, 1.0, OP.mult, OP.add)
                ab = wk.tile([P, 2, 2, W], BF16, tag="ab")
                nc.vector.tensor_mul(fl4(ab[:]), fl4(uv[:]), fl4(cpcm[:]))
                aabb = wk.tile([P, 2, W], BF16, tag="aabb")
                nc.vector.tensor_mul(fl3(aabb[:]),
                                     fl3(ab[:, 0]), fl3(ab[:, 1]))
                s1 = wk.tile([P, W], BF16, tag="s1")
                nc.vector.tensor_add(s1[:], aabb[:, 0, :], aabb[:, 1, :])
                g = tl.tile([P, W], BF16, tag=f"g{s}")
                nc.vector.scalar_tensor_tensor(g[:], ccs[s][:], 2.0, s1[:],
                                               OP.mult, OP.add)
                nc.vector.tensor_scalar(g[:], g[:], GMAX, None, OP.min)
                gbufs.append(g)

            # ---- tail: lns (one table set), exps, atans; d/sq/red on DVE
            lnps, lnqs, ds, rs, a_s = [], [], [], [], []
            for s in range(SPANS):
                lnp = tl.tile([P, W], BF16, tag=f"lnp{s}")
                tail_chain.append(nc.scalar.activation(
                    lnp[:], gbufs[s][:], AF.Ln, bias=b_p2[:], scale=1.0))
                lnps.append(lnp)
                lnq = tl.tile([P, W], BF16, tag=f"lnq{s}")
                tail_chain.append(nc.scalar.activation(
                    lnq[:], gbufs[s][:], AF.Ln, bias=b_p6[:], scale=-1.0))
                lnqs.append(lnq)
            for s in range(SPANS):
                d = tl.tile([P, W], BF16, tag=f"d{s}")
                nc.vector.tensor_sub(d[:], lnqs[s][:], lnps[s][:])
                ds.append(d)
            for s in range(SPANS):
                r = tl.tile([P, W], BF16, tag=f"r{s}")
                tail_chain.append(nc.scalar.activation(
                    r[:], ds[s][:], AF.Exp, bias=0.0, scale=0.5))
                rs.append(r)
            for s in range(SPANS):
                a = tl.tile([P, W], BF16, tag=f"a{s}")
                tail_chain.append(nc.scalar.activation(
                    a[:], rs[s][:], AF.Arctan))
                a_s.append(a)
            for s in range(SPANS):
                sq = tl.tile([P, W], BF16, tag=f"sq{s}")
                nc.vector.tensor_mul(sq[:], a_s[s][:], a_s[s][:])
                nc.vector.reduce_sum(acc[:, s:s + 1], sq[:],
                                     axis=mybir.AxisListType.X)

            # explicit natural_log_exp_and_others load so ln+exp share a set
            ld_nle = nc.scalar.add_instruction(mybir.InstLoadActFuncSet(
                name=nc.get_next_instruction_name(), ins=[], outs=[],
                act_func_set_id=6))

            # pin ACT order: sins (arrival) then ln*, exp*, atan* groups
            full = order + [ld_nle] + tail_chain
            def _raw(x):
                return x.ins if hasattr(x, "ins") else x
            for ai, bi in zip(full, full[1:]):
                add_dep_helper(_raw(bi), _raw(ai), sync=False,
                               reason="ACT table-set ordering")

            nc.sync.dma_start(out=out[:], in_=acc[:])

    nc.compile()
    return nc


_CACHE = {}


def _get_nc():
    if "nc" not in _CACHE:
        _CACHE["nc"] = build_bass()
    return _CACHE["nc"]


def _run(x, x_hat, **spmd_kwargs):
    x = np.ascontiguousarray(np.asarray(x, dtype=np.float32).reshape(3, NVOX))
    xh = np.ascontiguousarray(
        np.asarray(x_hat, dtype=np.float32).reshape(3, NVOX))
    wi = np.eye(P, dtype=np.float32)
    wni = -np.eye(P, dtype=np.float32)

    in_maps = []
    for c in range(N_CORES):
        sl = slice(c * PER, (c + 1) * PER)
        in_maps.append({
            "xs": np.ascontiguousarray(x[:, sl]),
            "xh": np.ascontiguousarray(xh[:, sl]),
            "wi": wi,
            "wni": wni,
        })

    nc = _get_nc()
    res = run_bass_kernel_spmd(
        nc, in_maps, core_ids=list(range(N_CORES)), **spmd_kwargs)
    total = 0.0
    for r in res.results:
        total += r["o"].astype(np.float64).sum()
    return np.float32(4.0 * total / NVOX), res


def kernel(x: np.ndarray, x_hat: np.ndarray) -> np.ndarray:
    val, _ = _run(x, x_hat)
    return val


# revision 9
# speedup vs baseline: 1.3653x; 1.0162x over previous
"""Trainium2 Bass kernel for nn_EulerMisorientation3D (v2).

reference math (per voxel, Bunge ZXZ Euler angles scaled by [2pi, pi, 2pi]):
    tr  = sum_i g_ii * g_hat_ii          (elementwise diag product)
    out = mean( arccos(0.5*(tr-1))^2 )

Closed form (alpha=2pi*x0, beta=pi*x1, gamma=2pi*x2):
    u = cos(2pi*s), s = x0+x2;  v = cos(2pi*t), t = x0-x2;  c = cos(pi*x1)
    P4 = 4(1+z) = 2 + 2*c*ch + A*Ah + B*Bh,  A = u*(1+c), B = v*(1-c)
    r  = sqrt(Q4/P4) = tan(theta/2),  Q4 = 8-P4 = 6-G,  G = P4-2
    theta = 2*atan(r),  r = exp(0.5*(ln Q4 - ln P4));  loss = mean(theta^2)

Half-angle trick kills all range wraps: u = 1-2*sin^2(pi*s) with
sin(pi*s-pi) arg within the sin spline domain even for x_hat noise
(|arg| <= ~1.24pi < 4), v = 1-2*sin^2(pi*t), c = -sin(pi*x1 - pi/2).

Engine split:
    PE    s = x0+x2 per tile via two identity-weight fp32r matmuls
          accumulating in PSUM (weights exactly 1 -> near-exact adds).
    Pool  t = x0-x2 per tile; cc = c*ch per span.
    ACT   6 sins/voxel then lnP, lnQ, exp(d/2), atan per span.  Queue is
          pinned in data-arrival order and grouped by table set:
          sin* (trig_and_small) -> ln* (natural_log) -> exp* -> atan*.
    DVE   bf16 chain with flat contiguous APs (2x/4x modes): q=sg^2,
          uv=1-2q, cp/cm, AB, AABB, S1, G=2cc+S1, clamp, d=lnQ-lnP,
          a^2, reduce.  theta^2 = 4*atan(r)^2; host multiplies by 4.

Sharding: voxel axis split over 8 cores; per-core acc[128, SPANS] partial
sums of atan(r)^2; host sums (f64) * 4 / N.
"""

import math

import numpy as np

import concourse.bacc as bacc
import concourse.tile as tile
from concourse.tile_rust import add_dep_helper
from concourse import mybir
from concourse.bass_utils import run_bass_kernel_spmd

F32 = mybir.dt.float32
F32R = mybir.dt.float32r
BF16 = mybir.dt.bfloat16
AF = mybir.ActivationFunctionType
OP = mybir.AluOpType

N_CORES = 8
NVOX = 128 * 128 * 128          # 2097152 voxels
PER = NVOX // N_CORES           # 262144 voxels per core
P = 128
COLS = PER // P                 # 2048
T = 4                           # tiles
C = COLS // T                   # 512
SPANS = T // 2                  # spans of 2 tiles (1024 cols)
W = 2 * C                       # span width

PI = math.pi
LN_EPS = 5e-5
GMAX = 5.99609375               # bf16-exact clamp: Q4 = 6+eps-G > 0


def build_bass():
    nc = bacc.Bacc("TRN2", target_bir_lowering=False, debug=False,
                   num_devices=N_CORES)
    xs = nc.declare_dram_parameter("xs", [3, PER], F32R, isOutput=False)
    xh = nc.declare_dram_parameter("xh", [3, PER], F32R, isOutput=False)
    wid = nc.declare_dram_parameter("wi", [P, P], F32R, isOutput=False)
    wnid = nc.declare_dram_parameter("wni", [P, P], F32R, isOutput=False)
    out = nc.declare_dram_parameter("o", [P, SPANS], F32, isOutput=True)

    xs_v = xs[:].rearrange("c (p q) -> p c q", p=P)   # [128, 3, 2048]
    xh_v = xh[:].rearrange("c (p q) -> p c q", p=P)

    with tile.TileContext(nc) as tc:
        with (
            tc.tile_pool(name="io", bufs=2) as io,
            tc.tile_pool(name="wk", bufs=2) as wk,
            tc.tile_pool(name="bt", bufs=2) as bt,
            tc.tile_pool(name="tl", bufs=1) as tl,
            tc.tile_pool(name="big", bufs=1) as big,
            tc.psum_pool(name="ps", bufs=2) as ps,
        ):
            acc = big.tile([P, SPANS], F32, tag="acc")
            b_mpi = big.tile([P, 1], F32, tag="b_mpi")
            b_mpi2 = big.tile([P, 1], F32, tag="b_mpi2")
            b_p2 = big.tile([P, 1], F32, tag="b_p2")
            b_p6 = big.tile([P, 1], F32, tag="b_p6")
            nc.vector.memset(b_mpi, -PI)
            nc.vector.memset(b_mpi2, -PI / 2)
            nc.vector.memset(b_p2, 2.0 + LN_EPS)
            nc.vector.memset(b_p6, 6.0 + LN_EPS)
            wi = big.tile([P, P], F32R, tag="wi")
            nc.sync.dma_start(out=wi, in_=wid[:])
            wni = big.tile([P, P], F32R, tag="wni")
            nc.sync.dma_start(out=wni, in_=wnid[:])

            # ---- tiles (allocated up-front; DMAs in arrival order)
            in4s = [io.tile([P, 2, 2, C], F32R, tag="in4", name=f"in4_{j}")
                    for j in range(T)]
            xbs = [bt.tile([P, 2, W], F32R, tag="xb", name=f"xb_{s}")
                   for s in range(SPANS)]

            def dma_tile(j):
                sl = slice(j * C, (j + 1) * C)
                nc.sync.dma_start(out=in4s[j][:, 0], in_=xs_v[:, 0:3:2, sl])
                nc.sync.dma_start(out=in4s[j][:, 1], in_=xh_v[:, 0:3:2, sl])

            def dma_beta(s):
                sl = slice(s * W, (s + 1) * W)
                nc.sync.dma_start(out=xbs[s][:, 0, :], in_=xs_v[:, 1, sl])
                nc.sync.dma_start(out=xbs[s][:, 1, :], in_=xh_v[:, 1, sl])

            dma_tile(0)
            dma_beta(0)
            dma_tile(1)
            dma_beta(1)
            dma_tile(2)
            dma_tile(3)

            # per-span tiles
            sgs = [wk.tile([P, 2, 2, W], BF16, tag="sg", name=f"sg_{s}")
                   for s in range(SPANS)]       # [x|h][sig_s|sig_t]
            sbs = [bt.tile([P, 2, W], BF16, tag="sb", name=f"sb_{s}")
                   for s in range(SPANS)]
            ccs = [bt.tile([P, W], BF16, tag="cc", name=f"cc_{s}")
                   for s in range(SPANS)]
            cpcms = [bt.tile([P, 2, 2, W], BF16, tag="cpcm",
                             name=f"cpcm_{s}") for s in range(SPANS)]

            sin_list = []    # ACT sins per tile, in emit order
            tail_chain = []  # ACT tail, grouped by table set

            # ---- per tile: all adds on PE (s and t) -> psum
            for j in range(T):
                in4 = in4s[j]
                pst = ps.tile([P, 2, C], F32, tag="pst")
                ptt = ps.tile([P, 2, C], F32, tag="ptt")
                for k in range(2):
                    nc.tensor.matmul(pst[:, k, :], wi[:], in4[:, k, 0, :],
                                     start=True, stop=False)
                    nc.tensor.matmul(pst[:, k, :], wi[:], in4[:, k, 1, :],
                                     start=False, stop=True)
                    nc.tensor.matmul(ptt[:, k, :], wi[:], in4[:, k, 0, :],
                                     start=True, stop=False)
                    nc.tensor.matmul(ptt[:, k, :], wni[:], in4[:, k, 1, :],
                                     start=False, stop=True)

                sp, h = j // 2, j % 2
                csl = slice(h * C, (h + 1) * C)
                sg = sgs[sp]
                sin_list.append(nc.scalar.activation(
                    sg[:, :, 0, csl], pst[:], AF.Sin,
                    bias=b_mpi[:], scale=PI))
                sin_list.append(nc.scalar.activation(
                    sg[:, :, 1, csl], ptt[:], AF.Sin, bias=0.0, scale=PI))

            # beta sins per span
            beta_sins = []
            for s in range(SPANS):
                ins = nc.scalar.activation(
                    sbs[s][:], xbs[s][:].bitcast(F32), AF.Sin,
                    bias=b_mpi2[:], scale=PI)
                beta_sins.append(ins)

            # ACT sin order: b0, s0, t0, s1, t1, b1, s2, t2, s3, t3
            order = [beta_sins[0], sin_list[0], sin_list[1],
                     sin_list[2], sin_list[3], beta_sins[1],
                     sin_list[4], sin_list[5], sin_list[6], sin_list[7]]

            # ---- per span: DVE product chain (flat APs for 2x/4x modes)
            gbufs = []
            for s in range(SPANS):
                sb, sg, cpcm = sbs[s], sgs[s], cpcms[s]
                for k in range(2):
                    nc.vector.tensor_scalar(cpcm[:, k, 0, :], sb[:, k, :],
                                            -1.0, 1.0, OP.mult, OP.add)
                    nc.vector.tensor_scalar(cpcm[:, k, 1, :], sb[:, k, :],
                                            1.0, None, OP.add)
                nc.vector.tensor_mul(ccs[s][:], sb[:, 0, :], sb[:, 1, :])
                fl4 = lambda ap: ap.rearrange("p a b w -> p (a b w)")
                fl3 = lambda ap: ap.rearrange("p a w -> p (a w)")
                qq = wk.tile([P, 2, 2, W], BF16, tag="qq")
                nc.vector.tensor_mul(fl4(qq[:]), fl4(sg[:]), fl4(sg[:]))
                uv = wk.tile([P, 2, 2, W], BF16, tag="uv")
                nc.vector.tensor_scalar(fl4(uv[:]), fl4(qq[:]),
                                        -2.0, 1.0, OP.mult, OP.add)
                ab = wk.tile([P, 2, 2, W], BF16, tag="ab")
                nc.vector.tensor_mul(fl4(ab[:]), fl4(uv[:]), fl4(cpcm[:]))
                aabb = wk.tile([P, 2, W], BF16, tag="aabb")
                nc.vector.tensor_mul(fl3(aabb[:]),
                                     fl3(ab[:, 0]), fl3(ab[:, 1]))
                s1 = wk.tile([P, W], BF16, tag="s1")
                nc.vector.tensor_add(s1[:], aabb[:, 0, :], aabb[:, 1, :])
                g = tl.tile([P, W], BF16, tag=f"g{s}")
                nc.vector.scalar_tensor_tensor(g[:], ccs[s][:], 2.0, s1[:],
                                               OP.mult, OP.add)
                nc.vector.tensor_scalar(g[:], g[:], GMAX, None, OP.min)
                gbufs.append(g)

            # ---- tail: lns (one table set), exps, atans; d/sq/red on DVE
            lnps, lnqs, ds, rs, a_s = [], [], [], [], []
            for s in range(SPANS):
                lnp = tl.tile([P, W], BF16, tag=f"lnp{s}")
                tail_chain.append(nc.scalar.activation(
                    lnp[:], gbufs[s][:], AF.Ln, bias=b_p2[:], scale=1.0))
                lnps.append(lnp)
                lnq = tl.tile([P, W], BF16, tag=f"lnq{s}")
                tail_chain.append(nc.scalar.activation(
                    lnq[:], gbufs[s][:], AF.Ln, bias=b_p6[:], scale=-1.0))
                lnqs.append(lnq)
            for s in range(SPANS):
                d = tl.tile([P, W], BF16, tag=f"d{s}")
                nc.vector.tensor_sub(d[:], lnqs[s][:], lnps[s][:])
                ds.append(d)
            for s in range(SPANS):
                r = tl.tile([P, W], BF16, tag=f"r{s}")
                tail_chain.append(nc.scalar.activation(
                    r[:], ds[s][:], AF.Exp, bias=0.0, scale=0.5))
                rs.append(r)
            for s in range(SPANS):
                a = tl.tile([P, W], BF16, tag=f"a{s}")
                tail_chain.append(nc.scalar.activation(
                    a[:], rs[s][:], AF.Arctan))
                a_s.append(a)
            sq_chain = []
            for s in range(SPANS):
                sq = tl.tile([P, W], BF16, tag=f"sq{s}")
                sq_chain.append(nc.scalar.activation(
                    sq[:], a_s[s][:], AF.Square, bias=0.0, scale=1.0,
                    accum_out=acc[:, s:s + 1]))

            # explicit table loads: trig at head, nle before lns, trig
            # again before atans (explicit loads avoid the implicit-load
            # pipeline drain)
            def _load(set_id):
                return nc.scalar.add_instruction(mybir.InstLoadActFuncSet(
                    name=nc.get_next_instruction_name(), ins=[], outs=[],
                    act_func_set_id=set_id))
            ld_trig0 = _load(9)
            ld_nle = _load(6)
            ld_trig1 = _load(9)

            # pin ACT order: sins (arrival) then ln*, exp*, atan* groups
            n_exp = len(tail_chain) - SPANS   # atans are the last SPANS
            full = ([ld_trig0] + order + [ld_nle] + tail_chain[:n_exp]
                    + [ld_trig1] + tail_chain[n_exp:] + sq_chain)
            def _raw(x):
                return x.ins if hasattr(x, "ins") else x
            for ai, bi in zip(full, full[1:]):
                add_dep_helper(_raw(bi), _raw(ai), sync=False,
                               reason="ACT table-set ordering")

            nc.sync.dma_start(out=out[:], in_=acc[:])

    nc.compile()
    return nc


_CACHE = {}


def _get_nc():
    if "nc" not in _CACHE:
        _CACHE["nc"] = build_bass()
    return _CACHE["nc"]


def _run(x, x_hat, **spmd_kwargs):
    x = np.ascontiguousarray(np.asarray(x, dtype=np.float32).reshape(3, NVOX))
    xh = np.ascontiguousarray(
        np.asarray(x_hat, dtype=np.float32).reshape(3, NVOX))
    wi = np.eye(P, dtype=np.float32)
    wni = -np.eye(P, dtype=np.float32)

    in_maps = []
    for c in range(N_CORES):
        sl = slice(c * PER, (c + 1) * PER)
        in_maps.append({
            "xs": np.ascontiguousarray(x[:, sl]),
            "xh": np.ascontiguousarray(xh[:, sl]),
            "wi": wi,
            "wni": wni,
        })

    nc = _get_nc()
    res = run_bass_kernel_spmd(
        nc, in_maps, core_ids=list(range(N_CORES)), **spmd_kwargs)
    total = 0.0
    for r in res.results:
        total += r["o"].astype(np.float64).sum()
    return np.float32(4.0 * total / NVOX), res


def kernel(x: np.ndarray, x_hat: np.ndarray) -> np.ndarray:
    val, _ = _run(x, x_hat)
    return val
